# revision 40
# baseline (speedup 1.0000x reference)
"""Trainium2 Bass kernel for nn_CapsuleLayer_31413390803000 (CapsuleLayer with
dynamic routing).

Mathematical collapse exploited
-------------------------------
The reference implements the classic CapsNet routing quirk: input_hat is the
same for every capsule k (the tf.matmul broadcast tiles W over k).  With b
initialised to zero:

  - iteration 0: softmax(0) = 1/16 exactly, so s[b,k,:] = colsum_n(h[b])/16,
    identical for all k; out rows are identical across k.
  - the agreement update b += h @ out^T is therefore constant along k, so
    softmax stays exactly uniform (exp(0)/16) for every later iteration.

Hence the whole 3-iteration routing reduces EXACTLY (bitwise in the
reference) to

  out[b, k, :] = squash( (sum_n inputs[b, n, :]) @ W / 16 )   for all k.

The kernel is therefore a memory-bound column-sum over N=1024 plus a tiny
[512] @ [512,256] matvec and the squash nonlinearity.

Sharding: data-parallel over batch B=16 across 8 cores (2 batches/core),
W replicated.  No cross-core communication.

Performance structure (measured on HW, steady state ~12.8us/iter =
~330 GB/s/core, at the measured pure-DMA floor of ~12.6us):
  - The 4 MiB/core input read bounds everything; all compute is hidden.
  - Stage-1 column-sum runs as float32r matmuls (1 PE cycle/row vs 4 for
    fp32; rel err ~2.5e-4, far inside the 2e-2 gate).
  - x loads alternate between the two HWDGE rings (SP + ACT) as one 2 MiB
    descriptor-fat DMA per batch; those engines carry ZERO compute ops,
    because any sem wait on a DMA-issuing engine head-of-line-blocks the
    x DMAs queued behind it (measured +1.7us/iter).
  - All scalar work (squash) runs on DVE, incl. rsqrt via quake seed + 2
    Newton steps (the ACT-only Sqrt activation would put a wait on the
    scalar ring).  SWDGE (gpsimd) DMA is avoided: DVE fp32 copies lock
    the shared port pair and starve its descriptor generation.
  - Emission is software-pipelined with per-stage lags (s1:0, transpose:
    +1, y/squash: +2, broadcast: +4, store: +5) so every cross-engine
    dependency is resolved ~a full iteration before the consuming engine
    reaches it.
"""

from contextlib import ExitStack

import numpy as np

import concourse.bass as bass  # noqa: F401
import concourse.tile as tile
from concourse import bacc, mybir
from concourse._compat import with_exitstack

# Problem shapes (hardcoded per contract).
B, N, DIN, KD = 16, 1024, 512, 256
NCAPS = 16
EPS = 1e-7
N_CORES = 8
B_LOC = B // N_CORES  # 2 batches per core

F32 = mybir.dt.float32
ACT = mybir.ActivationFunctionType


NCH = N // 128   # 8 n-chunks of 128 rows per batch
DCH = DIN // 128  # 4 din-chunks
CPD = 8          # n-chunks per DMA (2 MiB per dma_start; one DMA per batch)
QMAGIC = 0x5F3759DF  # quake fast-rsqrt seed (f32 bit trick)
# float32r truncates the multiply mantissa (measured ~1e-4..2.6e-4 rel
# err vs fp32's 6.6e-7) but runs the PE at 1 cycle/row instead of 4.
# The grading tolerance is 2e-2, so the 100x margin buys a 4x PE
# speedup and the kernel becomes purely DMA-bound.
F32R = mybir.dt.float32r


class _St:
    pass


def _setup(
    ctx: ExitStack,
    tc: "tile.TileContext",
    w,
    load_w_early: bool,
    rings=("sync", "scalar"),
    cpd: int = CPD,
    s1_f32: bool = False,
    store_rings=None,
    merge_store: bool = True,
    fuse_batches: bool = False,
    squash_engine: str = "vector",
):
    """Pools + constants + (optionally) the one-time W load."""
    st = _St()
    st.rings = rings
    st.cpd = cpd
    st.s1_f32 = s1_f32
    st.store_rings = store_rings or rings
    st.merge_store = merge_store
    st.fuse_batches = fuse_batches
    st.squash_engine = squash_engine
    nc = tc.nc
    # 3 reps of x lookahead either way: fused allocates one [128, B_LOC,
    # NCH, DIN] tile per rep, unfused B_LOC tiles per rep.
    st.xpool = ctx.enter_context(
        tc.tile_pool(name="xp", bufs=3 if fuse_batches else 3 * B_LOC)
    )
    st.wpool = ctx.enter_context(tc.tile_pool(name="wp", bufs=1))
    consts = ctx.enter_context(tc.tile_pool(name="cp", bufs=1))
    st.small = ctx.enter_context(tc.tile_pool(name="sp", bufs=6))
    st.outp = ctx.enter_context(tc.tile_pool(name="op", bufs=4))
    st.ps_v = ctx.enter_context(tc.tile_pool(name="ps_v", bufs=2, space="PSUM"))
    st.ps_t = ctx.enter_context(tc.tile_pool(name="ps_t", bufs=2, space="PSUM"))
    st.ps_y = ctx.enter_context(tc.tile_pool(name="ps_y", bufs=2, space="PSUM"))
    st.ps_o = ctx.enter_context(tc.tile_pool(name="ps_o", bufs=2, space="PSUM"))

    # Constants.
    st.ones_col_f32 = consts.tile([128, 1], F32)
    nc.vector.memset(st.ones_col_f32, 1.0 / NCAPS)
    st.ones_col = consts.tile([128, 1], F32R)  # 1/16 folds the uniform softmax
    nc.vector.tensor_copy(st.ones_col, st.ones_col_f32)
    st.one_one = consts.tile([1, 1], F32)
    nc.vector.memset(st.one_one, 1.0)
    st.ones_row = consts.tile([1, NCAPS], F32)
    nc.vector.memset(st.ones_row, 1.0)
    st.magic = consts.tile([1, 1], mybir.dt.int32)
    nc.vector.memset(st.magic, QMAGIC)
    st.c1p5 = consts.tile([1, 1], F32)
    nc.vector.memset(st.c1p5, 1.5)

    # W [512,256] -> SBUF [128, 4, 256] (din-chunk c at free index c).
    # Loaded once; in the unrolled body it is sequenced on the SP ring after
    # batch 0's x chunks so batch 0's loads arrive first.
    st.w_r = st.wpool.tile([128, DCH, KD], F32R)
    st.w_loaded = False
    st.dma_idx = 0  # ring rotation persists across reps
    if load_w_early:
        _load_w(st, tc, None, w)
    st.w_dram = w
    return st


def _load_w(st, tc, _x, w):
    tc.nc.sync.dma_start(
        out=st.w_r,
        in_=w.rearrange("(c p) d -> p c d", p=128).bitcast(F32R),
    )
    st.w_loaded = True


def _stage_load_s1(st, tc, x):
    """DMA both batches in, column-sum each on the PE, copy v out of PSUM.

    Row-to-partition map "(p a) d": partition p holds rows [8p, 8p+8) of
    the batch, so each partition's slice of a CPD-chunk DMA is CPD*DIN*4 =
    8 KiB CONTIGUOUS DRAM -> one fat descriptor per partition (the
    column-sum is row-permutation invariant, so any row->partition map is
    correct).
    """
    nc = tc.nc
    ctx = _St()
    cpd = st.cpd
    s1_dt = F32 if st.s1_f32 else F32R
    if st.fuse_batches:
        # One 4 MiB DMA for BOTH batches (x is batch-contiguous in DRAM);
        # whole-rep DMAs alternate between the two HWDGE rings, so each
        # ring streams a full rep while the other's is in flight.
        xf_t = st.xpool.tile([128, B_LOC, NCH, DIN], s1_dt, tag="x")
        eng = getattr(nc, st.rings[st.dma_idx % len(st.rings)])
        st.dma_idx += 1
        eng.dma_start(
            out=xf_t,
            in_=x.rearrange("b (p a) d -> p b a d", p=128).bitcast(s1_dt),
        )
        xtiles = [xf_t[:, b] for b in range(B_LOC)]
        if not st.w_loaded:
            _load_w(st, tc, x, st.w_dram)
    else:
        xtiles = []
        for b in range(B_LOC):
            xr = x[b].rearrange("(p a) d -> p a d", p=128)  # [128, NCH, DIN]
            xt = st.xpool.tile([128, NCH, DIN], s1_dt, tag="x")
            for c in range(NCH // cpd):
                eng = getattr(nc, st.rings[st.dma_idx % len(st.rings)])
                st.dma_idx += 1
                eng.dma_start(
                    out=xt[:, c * cpd:(c + 1) * cpd, :],
                    in_=xr[:, c * cpd:(c + 1) * cpd, :].bitcast(s1_dt),
                )
            xtiles.append(xt)
            if not st.w_loaded and b == 0:
                _load_w(st, tc, x, st.w_dram)

    # stage 1: v = (1/16) * colsum_n x[b] -> PSUM [1, DIN].  8 accumulating
    # f32r matmuls per batch (1 cycle/row on the PE; ~213ns per chunk).
    ones = st.ones_col_f32 if st.s1_f32 else st.ones_col
    v_pss = []
    for b in range(B_LOC):
        xt = xtiles[b]
        v_ps = st.ps_v.tile([1, DIN], F32, tag="v", name=f"v_{b}")
        for a in range(NCH):
            nc.tensor.matmul(
                v_ps,
                lhsT=ones,
                rhs=xt[:, a, :],
                start=(a == 0),
                stop=(a == NCH - 1),
            )
        v_pss.append(v_ps)
    # copy v out of PSUM immediately (DVE) so the PSUM bank frees fast and
    # downstream PE stages never wait on this copy.  (No ACT: the scalar
    # engine is a DMA ring and must carry NO compute, else its sem waits
    # head-of-line-block the x DMAs queued behind it.)
    ctx.v_sb = []
    for b in range(B_LOC):
        v_sb = st.small.tile([1, DIN], F32, tag="v_sb")
        nc.vector.tensor_copy(v_sb, v_pss[b])
        ctx.v_sb.append(v_sb)
    return ctx


def _stage_trans(st, tc, ctx):
    """Transpose v row -> [128, 4] columns via 4 tiny N=1 matmuls (plain
    fp32: f32r has a dst-pattern restriction at N=1), then copy to SBUF."""
    nc = tc.nc
    ctx.vt_sb = []
    for b in range(B_LOC):
        vt_sb = st.small.tile([128, DCH], F32R, tag="vt_sb")
        vt_ps = st.ps_t.tile([128, DCH], F32, tag="vt", name=f"vt_{b}")
        for dj in range(DCH):
            nc.tensor.matmul(
                vt_ps[:, dj:dj + 1],
                lhsT=ctx.v_sb[b][:, dj * 128:(dj + 1) * 128],
                rhs=st.one_one,
                start=True,
                stop=True,
            )
        nc.vector.tensor_copy(vt_sb, vt_ps)
        ctx.vt_sb.append(vt_sb)


def _stage_y(st, tc, ctx):
    """y = v @ W on the PE, then the squash scale chain — ALL on DVE (the
    scalar/sync engines are DMA rings and must carry no compute; the ACT
    sqrt is replaced by a quake-rsqrt seed + 2 Newton steps on DVE)."""
    nc = tc.nc
    MUL = mybir.AluOpType.mult
    I32 = mybir.dt.int32
    ctx.y_r, ctx.sc_row = [], []
    for b in range(B_LOC):
        y_ps = st.ps_y.tile([1, KD], F32, tag="y")
        for dj in range(DCH):
            nc.tensor.matmul(
                y_ps,
                lhsT=ctx.vt_sb[b][:, dj:dj + 1],
                rhs=st.w_r[:, dj, :],
                start=(dj == 0),
                stop=(dj == DCH - 1),
            )
        # f32r-rounded copy, used both as the stage-5 broadcast matmul rhs
        # (the BIR verifier requires f32r matmul inputs to be produced as
        # f32r, not bitcast) and — bitcast back — for the squash sum.
        y_r = st.small.tile([1, KD], F32R, tag="y_r")
        nc.vector.tensor_copy(y_r, y_ps)
        ctx.y_r.append(y_r)
        y_f = y_r.bitcast(F32)

        # squash scale = sq / ((1+sq) * sqrt(sq+eps)) = sq * rsqrt(sq+eps)
        # * (1/(1+sq)), with rsqrt via quake seed + 2 Newton iterations.
        # The whole scalar chain can run on DVE or on the otherwise-idle
        # GpSimd (st.squash_engine).
        se = getattr(nc, st.squash_engine)
        ysq = st.small.tile([1, KD], F32, tag="ysq")
        sq = st.small.tile([1, 1], F32, tag="sq")
        # accum_out lowers to a Ptr-variant opcode that Pool rejects; the
        # [1,KD] data op stays on DVE in either mode.
        nc.vector.scalar_tensor_tensor(
            ysq, y_f, 1.0, y_f, op0=MUL, op1=MUL, accum_out=sq
        )
        s_e = st.small.tile([1, 1], F32, tag="s_e")
        se.tensor_scalar_add(s_e, sq, EPS)  # s = sq + eps
        ti = st.small.tile([1, 1], I32, tag="ti")
        se.tensor_scalar(
            ti, s_e.bitcast(I32), 1, None, op0=mybir.AluOpType.logical_shift_right
        )
        y0i = st.small.tile([1, 1], I32, tag="y0i")
        se.tensor_sub(y0i, st.magic, ti)  # seed: magic - (i >> 1)
        yv = y0i.bitcast(F32)
        for it in range(2):  # Newton: y <- y * (1.5 - 0.5*s*y^2)
            yy = st.small.tile([1, 1], F32, tag=f"yy{it}")
            se.scalar_tensor_tensor(yy, yv, 1.0, yv, op0=MUL, op1=MUL)
            t1 = st.small.tile([1, 1], F32, tag=f"t1{it}")
            se.scalar_tensor_tensor(t1, s_e, 0.5, yy, op0=MUL, op1=MUL)
            f1 = st.small.tile([1, 1], F32, tag=f"f1{it}")
            se.scalar_tensor_tensor(
                f1, t1, -1.0, st.c1p5, op0=MUL, op1=mybir.AluOpType.add
            )
            yn = st.small.tile([1, 1], F32, tag=f"yn{it}")
            se.tensor_mul(yn, yv, f1)
            yv = yn
        d = st.small.tile([1, 1], F32, tag="d")
        se.tensor_scalar_add(d, sq, 1.0)
        rec = st.small.tile([1, 1], F32, tag="rec")
        if st.squash_engine == "vector":
            nc.vector.reciprocal(rec, d)  # 1/(1+sq)
        else:
            se.scalar_tensor_tensor(
                rec, st.one_one, 1.0, d, op0=MUL, op1=mybir.AluOpType.divide
            )
        t3 = st.small.tile([1, 1], F32, tag="t3")
        se.tensor_mul(t3, sq, yv)  # sq * rsqrt(sq+eps)
        sc = st.small.tile([1, 1], F32, tag="sc")
        se.tensor_mul(sc, t3, rec)
        sc_row = st.small.tile([1, NCAPS], F32R, tag="sc_row")
        # scalar-from-AP (TensorScalarPtr) is DVE-only; keep this one op
        # on vector even when the chain runs on gpsimd
        nc.vector.tensor_scalar_mul(sc_row, st.ones_row, sc)
        ctx.sc_row.append(sc_row)


def _stage_s5(st, tc, ctx):
    """out[k, :] = scale * y broadcast to 16 capsules (PE + DVE copy)."""
    nc = tc.nc
    if st.merge_store:
        # Both batches land in one SBUF tile [16, B_LOC, 256] so the store
        # is a single DMA (one ring slot + one completion instead of two).
        o_sb = st.outp.tile([NCAPS, B_LOC, KD], F32, tag="o_sb")
        ctx.o_sb = o_sb
    else:
        ctx.o_sb = []
    for b in range(B_LOC):
        o_ps = st.ps_o.tile([NCAPS, KD], F32, tag="o")
        nc.tensor.matmul(
            o_ps,
            lhsT=ctx.sc_row[b],
            rhs=ctx.y_r[b],
            start=True,
            stop=True,
        )
        if st.merge_store:
            nc.vector.tensor_copy(ctx.o_sb[:, b, :], o_ps)
        else:
            o_sb = st.outp.tile([NCAPS, KD], F32, tag="o_sb")
            nc.vector.tensor_copy(o_sb, o_ps)
            ctx.o_sb.append(o_sb)


def _stage_store(st, tc, ctx, o):
    """DRAM store, lagged one further slot so its sem wait is pre-resolved
    by the time the DMA ring reaches it (never blocks later x DMAs)."""
    nc = tc.nc
    if st.merge_store:
        # rotate with the shared counter so the extra 16 KiB alternates
        # between the two rings across reps
        eng = getattr(nc, st.store_rings[st.dma_idx % len(st.store_rings)])
        st.dma_idx += 1
        eng.dma_start(out=o.rearrange("b k d -> k b d"), in_=ctx.o_sb)
    else:
        for b in range(B_LOC):
            eng = getattr(nc, st.store_rings[b % len(st.store_rings)])
            eng.dma_start(out=o[b], in_=ctx.o_sb[b])


def _emit_pipeline(st, tc, x, o, reps: int):
    """Software-pipelined emission: each tail stage lags far enough that
    every wait is resolved before the owning engine reaches it — the PE
    FIFO never stalls on the DVE squash chain, and the two DMA rings
    (sync/scalar) only ever see pre-resolved sem waits."""
    ctxs = []
    for k in range(reps + 5):
        if k < reps:
            ctxs.append(_stage_load_s1(st, tc, x))
        if 0 <= k - 1 < reps:
            _stage_trans(st, tc, ctxs[k - 1])
        if 0 <= k - 2 < reps:
            _stage_y(st, tc, ctxs[k - 2])
        if 0 <= k - 4 < reps:
            _stage_s5(st, tc, ctxs[k - 4])
        if 0 <= k - 5 < reps:
            _stage_store(st, tc, ctxs[k - 5], o)
            ctxs[k - 5] = None  # release tile handles


@with_exitstack
def _capsule_body(ctx: ExitStack, tc: "tile.TileContext", x, w, o, repeats: int = 1):
    """Per-core kernel body (Python-unrolled repeats).

    x: [B_LOC, N, DIN] f32 DRAM in
    w: [DIN, KD]       f32 DRAM in
    o: [B_LOC, NCAPS, KD] f32 DRAM out
    repeats: re-execute the whole computation this many times (benchmarking
             only; results identical).
    """
    st = _setup(ctx, tc, w, load_w_early=False)
    _emit_pipeline(st, tc, x, o, repeats)


def make_loop_body(unroll: int = 16, **kw):
    """Body with a hardware For_i loop of ``repeats // unroll`` iterations,
    each containing ``unroll`` software-pipelined repetitions.  Keeps the
    NEFF small for very large repeat counts (benchmarking).  The all-engine
    barrier at the loop back-edge drains the pipeline each iteration, so
    the measured per-rep time overestimates steady state by roughly
    (fill+drain)/unroll."""

    @with_exitstack
    def body(ctx: ExitStack, tc: "tile.TileContext", x, w, o, repeats: int = 1):
        assert repeats % unroll == 0 and repeats >= unroll
        st = _setup(ctx, tc, w, load_w_early=True, **kw)
        with tc.For_i(0, repeats // unroll):
            _emit_pipeline(st, tc, x, o, unroll)

    return body


def _build_nc(repeats: int = 1, body=None):
    nc = bacc.Bacc(
        "TRN2",
        target_bir_lowering=False,
        debug=False,
        num_devices=N_CORES,
    )
    x = nc.dram_tensor("x", [B_LOC, N, DIN], F32, kind="ExternalInput")
    w = nc.dram_tensor("w", [DIN, KD], F32, kind="ExternalInput")
    o = nc.dram_tensor("o", [B_LOC, NCAPS, KD], F32, kind="ExternalOutput")
    with tile.TileContext(nc) as tc:
        (body or _capsule_body)(tc, x.ap(), w.ap(), o.ap(), repeats=repeats)
    nc.compile()
    return nc


class Runner:
    """Cached PJRT executor for the SPMD bass kernel (8 cores).

    Mirrors concourse.bass2jax.run_bass_via_pjrt's multi-core path, but
    keeps the jitted executable alive so repeated kernel() calls don't
    re-trace/re-lower.
    """

    def __init__(self, repeats: int = 1, body=None):
        import jax
        from jax.experimental.shard_map import shard_map
        from jax.sharding import Mesh, PartitionSpec

        from concourse import bass2jax

        bass2jax.install_neuronx_cc_hook()
        self.nc = _build_nc(repeats=repeats, body=body)
        nc = self.nc

        partition_name = (
            nc.partition_id_tensor.name if nc.partition_id_tensor else None
        )
        in_names, out_names, out_avals, zero_outs = [], [], [], []
        for alloc in nc.m.functions[0].allocations:
            if not isinstance(alloc, mybir.MemoryLocationSet):
                continue
            name = alloc.memorylocations[0].name
            if alloc.kind == "ExternalInput":
                if name != partition_name:
                    in_names.append(name)
            elif alloc.kind == "ExternalOutput":
                shape = tuple(alloc.tensor_shape)
                dtype = mybir.dt.np(alloc.dtype)
                out_names.append(name)
                out_avals.append(jax.core.ShapedArray(shape, dtype))
                zero_outs.append(np.zeros(shape, dtype))
        self.in_names = in_names
        self.out_names = out_names
        self.out_avals = out_avals
        self.zero_outs = zero_outs
        n_params = len(in_names)
        n_outs = len(out_avals)
        all_in_names = in_names + out_names
        if partition_name is not None:
            all_in_names.append(partition_name)

        def _body(*args):
            operands = list(args)
            if partition_name is not None:
                operands.append(bass2jax.partition_id_tensor())
            outs = bass2jax._bass_exec_p.bind(
                *operands,
                out_avals=tuple(out_avals),
                in_names=tuple(all_in_names),
                out_names=tuple(out_names),
                lowering_input_output_aliases=(),
                sim_require_finite=True,
                sim_require_nnan=True,
                nc=nc,
            )
            return tuple(outs)

        self._body = _body
        devices = jax.devices()[:N_CORES]
        assert len(devices) == N_CORES
        self.mesh = Mesh(np.asarray(devices), ("core",))
        in_specs = (PartitionSpec("core"),) * (n_params + n_outs)
        out_specs = (PartitionSpec("core"),) * n_outs
        self.jitted = jax.jit(
            shard_map(
                _body,
                mesh=self.mesh,
                in_specs=in_specs,
                out_specs=out_specs,
                check_rep=False,
            ),
            donate_argnums=tuple(range(n_params, n_params + n_outs)),
            keep_unused=True,
        )

    def concat_inputs(self, in_maps):
        return [
            np.concatenate([np.asarray(m[name]) for m in in_maps], axis=0)
            for name in self.in_names
        ]

    def concat_zeros(self):
        return [
            np.zeros((N_CORES * z.shape[0], *z.shape[1:]), z.dtype)
            for z in self.zero_outs
        ]

    def __call__(self, concat_in):
        out_arrs = self.jitted(*concat_in, *self.concat_zeros())
        return [np.asarray(a) for a in out_arrs]


_RUNNERS: dict = {}


def get_runner(repeats: int = 1) -> Runner:
    if repeats not in _RUNNERS:
        _RUNNERS[repeats] = Runner(repeats=repeats)
    return _RUNNERS[repeats]


def kernel(inputs: np.ndarray, W: np.ndarray):
    """Full-input entry point: shard over 8 cores, run, gather."""
    assert inputs.shape == (B, N, DIN) and W.shape == (DIN, KD)
    runner = get_runner(1)
    xf = np.ascontiguousarray(inputs, dtype=np.float32)
    wf = np.ascontiguousarray(W, dtype=np.float32)
    in_maps = [
        {"x": xf[c * B_LOC:(c + 1) * B_LOC], "w": wf} for c in range(N_CORES)
    ]
    outs = runner(runner.concat_inputs(in_maps))
    # single output "o": [N_CORES * B_LOC, NCAPS, KD] -> [B, NCAPS, KD]
    return outs[0].reshape(B, NCAPS, KD)

